# revision 37
# baseline (speedup 1.0000x reference)
"""Trainium2 Bass kernel for nn_Attention (LN -> QKV -> RoPE -> softmax attn -> out-proj).

Sharding: tensor-parallel over heads. Each of the 8 cores computes 2 of the 16
heads for both batches (column-split w_qkv, row-split w_out) and produces a
partial (DIM, B*N) output in transposed layout; the host sums the 8 partials
and adds b_out.

Device-side pipeline (single flat schedule, engines overlapped):
  per token-quarter (1024 tokens): LN stats via ones-column matmuls over x and
  host-precomputed x^2 (both bf16 inputs), stat math on DVE in f32,
  rsqrt = exp(-0.5*ln(var+eps)) on Act (stays inside the one natural_log_exp
  activation table -> no table reloads), per-token scale (f32r) broadcast via
  1-partition ones-row matmuls, raw QKV matmuls in bf16 with the LN mean
  correction fused into a scalar_tensor_tensor PSUM eviction (f32);
  per batch: RoPE in f32 on DVE (pair-swap stream_shuffle + cos/sin) with a
  single final rounding to bf16, V transposed on PE into per-head
  [keys, dh+ones] tiles; attention in the S^T orientation (scores
  [keys, queries]); softmax denominator rides the attnV matmul as the ones
  row; 1/denom = exp(-ln(x)) on Act; the whole per-chunk epilogue
  (denominator, normalize, out-proj, output DMA) is deferred into the next
  chunk's jt-loop so the PE never drains.
"""
import sys
sys.path.insert(0, "/opt/trn_rl_repo")

import numpy as np
import ml_dtypes
from contextlib import ExitStack

import bass_rust
import concourse.bass as bass
import concourse.tile as tile
from concourse import mybir

F32 = mybir.dt.float32
F32R = mybir.dt.float32r
BF16 = mybir.dt.bfloat16
FP8 = mybir.dt.float8e4
AF = mybir.ActivationFunctionType
OP = mybir.AluOpType

# ---------------------------------------------------------------------------
# walrus in this image rejects >1 sync-wait on a Drain (CTRL) instruction;
# split the TileContext epilogue drain into a chain of single-wait drains.
_orig_drain_and_barrier = tile.TileContext._drain_and_barrier


def _split_drain_and_barrier(self, tick_clock, wait_clock):
    from bass_rust import ScopedClock

    drain_inst = self.nc.sync.drain()
    wait_clock.add_sem_waits(drain_inst.ins, ScopedClock({None: tick_clock.global_clock}))
    waits = list(drain_inst.ins.sync_info.on_wait)
    if len(waits) > 1:
        ups = list(drain_inst.ins.sync_info.on_update)
        drain_inst.ins.sync_info = bass_rust.SyncInfo(on_wait=waits[:1], on_update=[])
        rest = waits[1:]
        while rest:
            chunk, rest = rest[:1], rest[1:]
            d2 = self.nc.sync.drain()
            d2.ins.sync_info = bass_rust.SyncInfo(
                on_wait=chunk, on_update=[] if rest else ups
            )
    self.nc.all_engine_barrier()
    assert self.sems is not None
    popped = self.nc._tile_sem_poison_stack.pop()
    assert popped is self._sem_poison
    self.nc.clear_and_free_semaphores(list(self.sems.allocated().values()))
    self.nc.all_engine_barrier()


tile.TileContext._drain_and_barrier = _split_drain_and_barrier

_WAIT_CAP = 1


def split_excess_waits(nc):
    """walrus in this image caps sync-waits per instruction very low. Move
    excess waits onto same-engine NOPs inserted immediately before the
    instruction (engine queues are in-order, so the gating is preserved)."""
    nid = [0]

    def mk_nop(engine, waits):
        nid[0] += 1
        n = bass_rust.InstNoOp(name=f"WSPL-{nid[0]}", engine=engine, ins=[], outs=[])
        n.sync_info = bass_rust.SyncInfo(on_wait=waits, on_update=[])
        return n

    for f in nc.m.functions:
        for bb in f.blocks:
            out = []
            for inst in bb.instructions:
                si = inst.sync_info
                waits = list(si.on_wait) if si is not None else []
                if len(waits) > _WAIT_CAP:
                    keep = waits[: _WAIT_CAP]
                    rest = waits[_WAIT_CAP:]
                    while rest:
                        chunk, rest = rest[:_WAIT_CAP], rest[_WAIT_CAP:]
                        out.append(mk_nop(inst.engine, chunk))
                    inst.sync_info = bass_rust.SyncInfo(
                        on_wait=keep, on_update=list(si.on_update))
                out.append(inst)
            bb.instructions = out


# ---------------------------------------------------------------------------
class Cfg:
    def __init__(self, DIM=1024, NB=2, NPB=2048, DH=64, H=2, IC=512, eps=1e-5):
        self.DIM, self.NB, self.NPB, self.DH, self.H = DIM, NB, NPB, DH, H
        self.TOK = NB * NPB
        self.KC = DIM // 128          # k-chunks of the QKV contraction
        self.QC = H * DH              # q/k/v columns per core (128)
        self.FC = 512                 # free chunk for matmuls
        self.QT = 1024                # tokens per quarter
        self.NQ = self.TOK // self.QT
        self.JT = NPB // 128          # key tiles per batch
        self.IC = IC                  # query chunk
        self.ICN = NPB // IC
        self.DO = DIM                 # out-proj output dim
        self.DOT = DIM // 128
        self.eps = eps
        assert self.QC == 128 and DIM % 128 == 0 and NPB % 128 == 0
        assert self.QT % self.FC == 0 and NPB % IC == 0 and NPB % self.QT == 0


def build_nc(c: Cfg, split_waits: bool = True):
    nc = bass.Bass("TRN2", target_bir_lowering=False)

    xt_d = nc.dram_tensor("xt", [128, c.KC, c.TOK], BF16, kind="ExternalInput")
    xsq_d = nc.dram_tensor("xsq", [128, c.KC, c.TOK], FP8, kind="ExternalInput")
    wq_d = nc.dram_tensor("wq", [128, c.KC, 3 * c.QC], BF16, kind="ExternalInput")
    fixc_d = nc.dram_tensor("fixc", [128, 6], F32, kind="ExternalInput")
    cosk_d = nc.dram_tensor("cosk", [128, c.NPB], BF16, kind="ExternalInput")
    sink_d = nc.dram_tensor("sink", [128, c.NPB], BF16, kind="ExternalInput")
    wout_d = nc.dram_tensor("wout", [128, c.DO], F32R, kind="ExternalInput")
    idblk_d = nc.dram_tensor("idblk", [128, c.DH], BF16, kind="ExternalInput")
    out_d = nc.dram_tensor("out", [128, c.DOT, c.TOK], F32, kind="ExternalOutput")

    FC = c.FC
    pairswap = [i ^ 1 for i in range(32)]

    with ExitStack() as ctx:
        tc = ctx.enter_context(tile.TileContext(nc))
        wp = ctx.enter_context(tc.tile_pool(name="wp", bufs=1))
        r1 = ctx.enter_context(tc.tile_pool(name="r1", bufs=1))
        rb = ctx.enter_context(tc.tile_pool(name="rb", bufs=2))
        r3 = ctx.enter_context(tc.tile_pool(name="r3", bufs=3))
        pp = ctx.enter_context(tc.tile_pool(name="pp", bufs=2, space="PSUM"))

        xts, xqs = {}, {}

        def dma_quarter(q):
            t = rb.tile([128, c.KC, c.QT], BF16, tag="xt", name=f"xtq{q}")
            s = rb.tile([128, c.KC, c.QT], FP8, tag="xq", name=f"xqq{q}")
            for i in range(4):
                kcs = slice(i * (c.KC // 4), (i + 1) * (c.KC // 4))
                tsl = slice(q * c.QT, (q + 1) * c.QT)
                nc.sync.dma_start(t[:, kcs, :], xt_d[:, kcs, tsl])
                nc.sync.dma_start(s[:, kcs, :], xsq_d[:, kcs, tsl])
            xts[q], xqs[q] = t, s

        dma_quarter(0)
        dma_quarter(1)
        wq = wp.tile([128, c.KC, 3 * c.QC], BF16)
        nc.sync.dma_start(wq[:], wq_d[:])
        ones_c = wp.tile([128, 1], BF16)
        nc.vector.memset(ones_c[:], 1.0)
        ones_8 = wp.tile([128, 1], FP8)
        nc.vector.memset(ones_8[:], 1.0)
        onesf = wp.tile([1, 128], F32)
        nc.vector.memset(onesf[:], 1.0)
        ones_r = wp.tile([1, 128], F32R)
        nc.vector.tensor_copy(ones_r[:], onesf[:])
        sel2f = wp.tile([1, 2, 128], F32)
        nc.vector.memset(sel2f[:], 0.0)
        nc.vector.memset(sel2f[:, 0, 0:64], 1.0)
        nc.vector.memset(sel2f[:, 1, 64:128], 1.0)
        sel2 = wp.tile([1, 2, 128], F32R)
        nc.vector.tensor_copy(sel2[:], sel2f[:])
        fixc = wp.tile([128, 6], F32)
        nc.sync.dma_start(fixc[:], fixc_d[:])
        cosk = wp.tile([128, c.NPB], BF16)
        nc.sync.dma_start(cosk[:], cosk_d[:])
        sink = wp.tile([128, c.NPB], BF16)
        nc.sync.dma_start(sink[:], sink_d[:])
        wout = wp.tile([128, c.DO], F32R)
        nc.sync.dma_start(wout[:], wout_d[:])
        idblk = wp.tile([128, c.DH], BF16)
        nc.sync.dma_start(idblk[:], idblk_d[:])

        bstate = {}
        pend = [None]   # deferred per-chunk epilogue state

        from collections import deque
        fillers = deque()

        def unit(fn):
            fillers.append(fn)

        def pump(n=2):
            for _ in range(min(n, len(fillers))):
                fillers.popleft()()

        def drain():
            while fillers:
                fillers.popleft()()

        def queue_stats(q):
            b, qt = divmod(q, 2)

            def alloc_batch():
                st = {
                    "qfix": r1.tile([128, c.NPB], F32, tag="qfix", name=f"qfix{b}"),
                    "kfix": r1.tile([128, c.NPB], F32, tag="kfix", name=f"kfix{b}"),
                    "vfix": r1.tile([128, c.NPB], BF16, tag="vfix", name=f"vfix{b}"),
                    "qbf": rb.tile([128, c.NPB], BF16, tag="qbf", name=f"qbf{b}"),
                    "kbf": rb.tile([128, c.NPB], BF16, tag="kbf", name=f"kbf{b}"),
                    "sb": [None, None], "mb": [None, None],
                }
                bstate[b] = st

            def alloc_stat():
                st = bstate[b]
                mun_q = rb.tile([1, c.QT], F32R, tag="mun", name=f"mun{q}")
                srow_q = rb.tile([1, c.QT], F32R, tag="srow", name=f"srow{q}")
                st[f"stat{qt}"] = (srow_q, mun_q)

            if qt == 0:
                unit(alloc_batch)
            unit(alloc_stat)

            def stat_mm(fc, which):
                def go():
                    xt_t, xq_t = xts[q], xqs[q]
                    fsl = slice(fc * FC, (fc + 1) * FC)
                    if which == 0:
                        s1P = pp.tile([1, FC], F32, tag="mm", name="s1P")
                        for kc in range(c.KC):
                            nc.tensor.matmul(s1P[:], ones_c[:], xt_t[:, kc, fsl],
                                             start=(kc == 0), stop=(kc == c.KC - 1))
                        bstate[b][f"s1P{qt}{fc}"] = s1P
                    else:
                        s2P = pp.tile([1, FC], F32, tag="mm", name="s2P")
                        for kc in range(c.KC):
                            nc.tensor.matmul(s2P[:], ones_8[:], xq_t[:, kc, fsl],
                                             start=(kc == 0), stop=(kc == c.KC - 1))
                        bstate[b][f"s2P{qt}{fc}"] = s2P
                return go

            def stat_math(fc):
                def go():
                    st = bstate[b]
                    srow_q, mun_q = st[f"stat{qt}"]
                    s1P, s2P = st[f"s1P{qt}{fc}"], st[f"s2P{qt}{fc}"]
                    fsl = slice(fc * FC, (fc + 1) * FC)
                    mun = mun_q[:, fsl]
                    nc.vector.tensor_scalar_mul(mun, s1P[:], -1.0 / c.DIM)
                    ve = r1.tile([1, FC], F32, tag="ve", name="ve")
                    nc.vector.tensor_scalar(ve[:], s2P[:], 1.0 / c.DIM, c.eps,
                                            OP.mult, OP.add)
                    m2 = r1.tile([1, FC], F32, tag="m2", name="m2")
                    nc.vector.tensor_tensor(m2[:], mun.bitcast(F32),
                                            mun.bitcast(F32), OP.mult)
                    vef = r1.tile([1, FC], F32, tag="vef", name="vef")
                    nc.vector.scalar_tensor_tensor(vef[:], m2[:], -1.0, ve[:],
                                                   OP.mult, OP.add)
                    lnv = r1.tile([1, FC], F32, tag="lnv", name="lnv")
                    nc.scalar.activation(lnv[:], vef[:], AF.Ln)
                    nc.scalar.activation(srow_q[:, fsl], lnv[:], AF.Exp,
                                         scale=-0.5)
                return go

            for fc in range(2):
                unit(stat_mm(fc, 0))
                unit(stat_mm(fc, 1))
                unit(stat_math(fc))

        def queue_bcqkv(q):
            b, qt = divmod(q, 2)

            def bc_unit():
                st = bstate[b]
                srow_q, mun_q = st[f"stat{qt}"]
                s_b = rb.tile([128, c.QT], F32, tag="sb", name=f"sb{q}")
                m_b = rb.tile([128, c.QT], F32, tag="mb", name=f"mb{q}")
                st["sb"][qt], st["mb"][qt] = s_b, m_b
                for fc in range(2):
                    fsl = slice(fc * FC, (fc + 1) * FC)
                    bcp = pp.tile([128, FC], F32, tag="mm", name="bcs")
                    nc.tensor.matmul(bcp[:], ones_r[:], srow_q[:, fsl],
                                     start=True, stop=True)
                    nc.scalar.copy(s_b[:, fsl], bcp[:])
                    bcm = pp.tile([128, FC], F32, tag="mm", name="bcm")
                    nc.tensor.matmul(bcm[:], ones_r[:], mun_q[:, fsl],
                                     start=True, stop=True)
                    nc.scalar.copy(m_b[:, fsl], bcm[:])

            unit(bc_unit)

            def qkv_unit(fc, m):
                def go():
                    st = bstate[b]
                    xt_t = xts[q]
                    m_b = st["mb"][qt]
                    fsl = slice(fc * FC, (fc + 1) * FC)
                    gsl = slice(qt * c.QT + fc * FC, qt * c.QT + (fc + 1) * FC)
                    dst = (st["qfix"], st["kfix"], st["vfix"])[m]
                    qp = pp.tile([128, FC], F32, tag="mm", name="qp")
                    for kc in range(c.KC):
                        nc.tensor.matmul(qp[:], wq[:, kc, m * 128:(m + 1) * 128],
                                         xt_t[:, kc, fsl],
                                         start=(kc == 0), stop=(kc == c.KC - 1))
                    nc.vector.scalar_tensor_tensor(dst[:, gsl], m_b[:, fsl],
                                                   fixc[:, m:m + 1], qp[:],
                                                   OP.mult, OP.add)
                return go

            for fc in range(2):
                for m in range(3):
                    unit(qkv_unit(fc, m))

            def vscale_unit():
                st = bstate[b]
                qsl = slice(qt * c.QT, (qt + 1) * c.QT)
                nc.vector.tensor_tensor(st["vfix"][:, qsl], st["vfix"][:, qsl],
                                        st["sb"][qt][:], OP.mult)

            unit(vscale_unit)

        def queue_batch_prep(b, half, what="all"):
            def alloc_unit():
                st = bstate[b]
                vaug = rb.tile([128, c.H, c.JT, c.DH + 1], BF16, tag="vaug",
                               name=f"vaug{b}")
                st["vaug"] = vaug
                nc.vector.memset(vaug[:, :, :, c.DH], 1.0)
                st["ctx"] = rb.tile([128, c.NPB], F32R, tag="ctx", name=f"ctx{b}")

            if half == 0 and what in ("vt", "all"):
                unit(alloc_unit)

            # V transpose into per-head [keys, dh+ones] tiles (ones row last
            # gives the softmax denominator as row DH of the attnV PSUM tile)
            def vt_unit(hh, half):
                def go():
                    st = bstate[b]
                    vfix, vaug = st["vfix"], st["vaug"]
                    dsl = slice(hh * c.DH, (hh + 1) * c.DH)
                    for jt in range(half * 8, half * 8 + 8):
                        tp = pp.tile([128, c.DH], BF16, tag="mm", name="vt")
                        nc.tensor.transpose(tp[:],
                                            vfix[dsl, jt * 128:(jt + 1) * 128],
                                            idblk[dsl, :])
                        if jt % 2 == 0:
                            nc.scalar.copy(vaug[:, hh, jt, 0:c.DH], tp[:])
                        else:
                            nc.vector.tensor_copy(vaug[:, hh, jt, 0:c.DH], tp[:])
                return go



            # RoPE in f32; the per-token LN scale and the bf16 rounding ride
            # the last op. (rotation commutes with the per-token scale)
            def rope_unit(which, hq):
                def go():
                    st = bstate[b]
                    src = st["qfix"] if which == 0 else st["kfix"]
                    dst = st["qbf"] if which == 0 else st["kbf"]
                    hsl = slice(hq * c.QT, (hq + 1) * c.QT)
                    sh = r1.tile([128, c.QT], F32, tag="sh", name="sh")
                    nc.vector.stream_shuffle(sh[:], src[:, hsl], pairswap)
                    nc.vector.tensor_tensor(src[:, hsl], src[:, hsl],
                                            cosk[:, hsl], OP.mult)
                    nc.vector.tensor_tensor(sh[:], sh[:], sink[:, hsl], OP.mult)
                    nc.vector.tensor_tensor(src[:, hsl], src[:, hsl], sh[:],
                                            OP.add)
                    nc.vector.tensor_tensor(dst[:, hsl], src[:, hsl],
                                            st["sb"][hq][:], OP.mult)
                return go

            if what in ("vt", "all"):
                unit(vt_unit(0, half))
                unit(vt_unit(1, half))
            if what in ("rope", "all"):
                unit(rope_unit(0, half))
                unit(rope_unit(1, half))

        def flush_pend_stage(stage):
            """Deferred epilogue of the previous query chunk, staged into the
            current chunk's jt loop so the PE never waits on the Act chain."""
            if pend[0] is None:
                return
            p = pend[0]
            if stage == "dlr":        # 1/denominator on Act (ln then exp)
                dl = r1.tile([1, 2 * c.IC], F32, tag="dl", name="dl")
                nc.scalar.activation(dl[:], p["ds"][:], AF.Ln)
                dr = r1.tile([1, 2 * c.IC], F32R, tag="dr", name="dr")
                nc.scalar.activation(dr[:], dl[:], AF.Exp, scale=-1.0)
                p["dr"] = dr
            elif stage == "rp":       # broadcast 1/denom over inner rows
                rp = pp.tile([128, c.IC], F32, tag="mm", name="rp")
                for hh in range(2):
                    nc.tensor.matmul(rp[:], sel2[:, hh, :],
                                     p["dr"][:, hh * c.IC:(hh + 1) * c.IC],
                                     start=(hh == 0), stop=(hh == 1))
                rps = rb.tile([128, c.IC], F32, tag="rps", bufs=1, name="rps")
                nc.vector.tensor_copy(rps[:], rp[:])
                # normalize the raw-evicted context in place
                ctx, isl = p["ctx"], p["isl"]
                nc.vector.tensor_tensor(ctx[0:64, isl],
                                        ctx[0:64, isl].bitcast(F32),
                                        rps[0:64, :], OP.mult)
                nc.vector.tensor_tensor(ctx[64:128, isl],
                                        ctx[64:128, isl].bitcast(F32),
                                        rps[64:128, :], OP.mult)
            elif isinstance(stage, int):   # out-proj, 2 dim-blocks per call
                ctx, isl = p["ctx"], p["isl"]
                hoff = p["b"] * c.NPB + p["ic"] * c.IC
                for mt in range(stage * 2, stage * 2 + 2):
                    yp = pp.tile([128, c.IC], F32, tag="mm", name="yp")
                    nc.tensor.matmul(yp[:], wout[:, mt * 128:(mt + 1) * 128],
                                     ctx[:, isl], start=True, stop=True)
                    yt = r3.tile([128, c.IC], F32, tag="yt", bufs=2, name="yt")
                    nc.vector.tensor_copy(yt[:], yp[:])
                    nc.sync.dma_start(out_d[:, mt, hoff:hoff + c.IC], yt[:])
                if stage == 3:
                    pend[0] = None

        def emit_attention(b, ic):
            st = bstate[b]
            qbf, kbf, vaug, ctx = st["qbf"], st["kbf"], st["vaug"], st["ctx"]
            isl = slice(ic * c.IC, (ic + 1) * c.IC)
            # free the previous chunk's attnV accumulators: raw-evict context
            # (normalization happens later, off the critical path)
            if pend[0] is not None:
                p = pend[0]
                nc.vector.tensor_copy(p["ctx"][0:64, p["isl"]], p["cps0"][0:64, :])
                nc.vector.tensor_copy(p["ctx"][64:128, p["isl"]], p["cps1"][0:64, :])
            cps0 = pp.tile([128, c.IC], F32, tag="c0", bufs=1, name="cps0")
            cps1 = pp.tile([128, c.IC], F32, tag="c1", bufs=1, name="cps1")

            def emit_scores(jt):
                jsl = slice(jt * 128, (jt + 1) * 128)
                sp = pp.tile([128, 2 * c.IC], F32, tag="sp", name="sp")
                nc.tensor.matmul(sp[:, 0:c.IC], kbf[0:64, jsl], qbf[0:64, isl],
                                 start=True, stop=True, tile_position=(0, 0))
                nc.tensor.matmul(sp[:, c.IC:], kbf[64:128, jsl],
                                 qbf[64:128, isl],
                                 start=True, stop=True, tile_position=(64, 0))
                return sp

            sps = {0: emit_scores(0), 1: emit_scores(1)}
            for jt in range(c.JT):
                sp = sps.pop(jt)
                e = r3.tile([128, 2 * c.IC], BF16, tag="e", bufs=2, name="e")
                nc.scalar.activation(e[:], sp[:], AF.Exp)
                if jt == 1:
                    flush_pend_stage("dlr")
                if jt + 2 < c.JT:
                    sps[jt + 2] = emit_scores(jt + 2)
                if jt == 2:
                    flush_pend_stage("rp")
                elif 3 <= jt <= 6:
                    flush_pend_stage(jt - 3)
                nc.tensor.matmul(cps0[0:c.DH + 1, :], vaug[:, 0, jt, :],
                                 e[:, 0:c.IC], start=(jt == 0), stop=(jt == c.JT - 1))
                nc.tensor.matmul(cps1[0:c.DH + 1, :], vaug[:, 1, jt, :],
                                 e[:, c.IC:], start=(jt == 0), stop=(jt == c.JT - 1))
                if 7 <= jt <= 13:
                    pump(3)
                elif jt < 7 and pend[0] is None:
                    pump(2)
            # softmax denominators out of the accumulators (row DH)
            ds = r1.tile([1, 2 * c.IC], F32, tag="ds", name="ds")
            nc.vector.tensor_copy(ds[:, 0:c.IC], cps0[c.DH:c.DH + 1, :])
            nc.vector.tensor_copy(ds[:, c.IC:], cps1[c.DH:c.DH + 1, :])
            pend[0] = {"b": b, "ic": ic, "isl": isl, "ctx": ctx,
                       "cps0": cps0, "cps1": cps1, "ds": ds}

        def flush_tail():
            p = pend[0]
            nc.vector.tensor_copy(p["ctx"][0:64, p["isl"]], p["cps0"][0:64, :])
            nc.vector.tensor_copy(p["ctx"][64:128, p["isl"]], p["cps1"][0:64, :])
            flush_pend_stage("dlr")
            flush_pend_stage("rp")
            for s in range(4):
                flush_pend_stage(s)

        # ---- global schedule ----
        # Lead-in: quarter-0 chain, with quarter-1 stat matmuls filling the
        # PE while quarter-0's stat math runs on DVE/Act; batch-0 half-0
        # RoPE/V-prep lands before attention starts. Everything for the
        # second half / second batch is pumped as filler units inside the
        # attention jt-loops so the PE never drains.
        queue_stats(0)
        queue_stats(1)
        drain()
        queue_bcqkv(0)
        drain()
        queue_batch_prep(0, 0, "vt")
        queue_bcqkv(1)
        queue_batch_prep(0, 0, "rope")
        queue_batch_prep(0, 1, "vt")
        queue_batch_prep(0, 1, "rope")
        drain()
        unit(lambda: dma_quarter(2))
        queue_stats(2)
        queue_bcqkv(2)
        queue_batch_prep(1, 0, "vt")
        queue_batch_prep(1, 0, "rope")
        unit(lambda: dma_quarter(3))
        queue_stats(3)
        queue_bcqkv(3)
        queue_batch_prep(1, 1, "vt")
        queue_batch_prep(1, 1, "rope")
        for ic in range(c.ICN):
            emit_attention(0, ic)
        drain()
        for ic in range(c.ICN):
            emit_attention(1, ic)
        flush_tail()

    if split_waits:
        split_excess_waits(nc)
    nc.finalize()
    return nc


# ---------------------------------------------------------------------------
# host side
def host_inputs(c: Cfg, core: int, x, ln_w, ln_b, w_qkv, w_out, b_out):
    """Build the per-core input dict (all numpy, layouts described in build_nc)."""
    DIM, DH, H = c.DIM, c.DH, c.H
    INNER = w_qkv.shape[1] // 3
    TOK = c.TOK
    bf = ml_dtypes.bfloat16
    f32 = np.float32
    xf = x.reshape(TOK, DIM)
    xt = np.ascontiguousarray(xf.T.reshape(c.KC, 128, TOK).transpose(1, 0, 2))
    xtb = xt.astype(bf)
    xsq = (xtb.astype(np.float32) ** 2).astype(ml_dtypes.float8_e4m3)

    cs = core * c.QC
    sc = DH ** -0.5
    wsl = np.concatenate([w_qkv[:, cs:cs + c.QC] * sc,
                          w_qkv[:, INNER + cs:INNER + cs + c.QC],
                          w_qkv[:, 2 * INNER + cs:2 * INNER + cs + c.QC]], axis=1)
    wq = (ln_w[:, None] * wsl).reshape(c.KC, 128, 3 * c.QC).transpose(1, 0, 2)
    u = ln_w @ wsl   # [384]
    v = ln_b @ wsl
    fixc = np.concatenate([u.reshape(3, c.QC).T, v.reshape(3, c.QC).T], axis=1)

    inv = np.exp(np.arange(0, DH, 2, dtype=np.float64) * (-np.log(10000.0) / DH))
    ang = np.arange(c.NPB, dtype=np.float64)[:, None] * inv[None, :]
    cosR = np.repeat(np.cos(ang), 2, axis=1)
    sinR = np.repeat(np.sin(ang), 2, axis=1)
    sign = np.tile([-1.0, 1.0], DH // 2)
    sinS = sinR * sign[None, :]
    cosk = np.tile(cosR.T, (H, 1))
    sink = np.tile(sinS.T, (H, 1))

    wout = w_out[core * 128:(core + 1) * 128, :]
    idblk = np.zeros((128, DH), np.float32)
    for i in range(DH):
        idblk[i, i] = 1.0
        idblk[DH + i, i] = 1.0

    return {
        "xt": xtb, "xsq": xsq, "wq": wq.astype(bf), "fixc": fixc.astype(f32),
        "cosk": cosk.astype(bf), "sink": sink.astype(bf),
        "wout": wout.astype(f32), "idblk": idblk.astype(bf),
    }


def assemble_output(c: Cfg, outs, B, N, b_out=None):
    yT = np.zeros((c.DO, c.TOK), np.float64)
    for o in outs:
        yT += o.transpose(1, 0, 2).reshape(c.DO, c.TOK).astype(np.float64)
    y = np.ascontiguousarray(yT.T.reshape(B, N, c.DIM))
    if b_out is not None:
        y += b_out.astype(np.float64)
    return y.astype(np.float32)


_NC_CACHE = {}


def kernel(x, ln_w, ln_b, w_qkv, w_out, b_out):
    from concourse.bass_utils import run_bass_kernel_spmd

    x = np.asarray(x, np.float32)
    ln_w = np.asarray(ln_w, np.float32)
    ln_b = np.asarray(ln_b, np.float32)
    w_qkv = np.asarray(w_qkv, np.float32)
    w_out = np.asarray(w_out, np.float32)
    b_out = np.asarray(b_out, np.float32)
    assert np.allclose(ln_b, 0.0), "device fast-path assumes ln_b == 0"

    B, N, DIM = x.shape
    c = Cfg(DIM=DIM, NB=B, NPB=N)
    key = (DIM, B, N)
    if key not in _NC_CACHE:
        _NC_CACHE[key] = build_nc(c)
    nc = _NC_CACHE[key]
    in_maps = [host_inputs(c, core, x, ln_w, ln_b, w_qkv, w_out, b_out)
               for core in range(8)]
    import time as _time
    last = None
    for attempt in range(3):
        try:
            res = run_bass_kernel_spmd(nc, in_maps, core_ids=list(range(8)))
            break
        except Exception as e:  # transient device-unrecoverable wedges recover on retry
            last = e
            _time.sleep(15)
    else:
        raise last
    return assemble_output(c, [res.results[cc]["out"] for cc in range(8)], B, N, b_out)


# revision 40
# speedup vs baseline: 1.0266x; 1.0266x over previous
"""Trainium2 Bass kernel for nn_Attention (LN -> QKV -> RoPE -> softmax attn -> out-proj).

Sharding: tensor-parallel over heads. Each of the 8 cores computes 2 of the 16
heads for both batches (column-split w_qkv, row-split w_out) and produces a
partial (DIM, B*N) output in transposed layout; the host sums the 8 partials
and adds b_out.

Device-side pipeline (single flat schedule, engines overlapped):
  per token-quarter (1024 tokens): LN stats via ones-column matmuls over x and
  host-precomputed x^2 (both bf16 inputs), stat math on DVE in f32,
  rsqrt = exp(-0.5*ln(var+eps)) on Act (stays inside the one natural_log_exp
  activation table -> no table reloads), per-token scale (f32r) broadcast via
  1-partition ones-row matmuls, raw QKV matmuls in bf16 with the LN mean
  correction fused into a scalar_tensor_tensor PSUM eviction (f32);
  per batch: RoPE in f32 on DVE (pair-swap stream_shuffle + cos/sin) with a
  single final rounding to bf16, V transposed on PE into per-head
  [keys, dh+ones] tiles; attention in the S^T orientation (scores
  [keys, queries]); softmax denominator rides the attnV matmul as the ones
  row; 1/denom = exp(-ln(x)) on Act; the whole per-chunk epilogue
  (denominator, normalize, out-proj, output DMA) is deferred into the next
  chunk's jt-loop so the PE never drains.
"""
import sys
sys.path.insert(0, "/opt/trn_rl_repo")

import numpy as np
import ml_dtypes
from contextlib import ExitStack

import bass_rust
import concourse.bass as bass
import concourse.tile as tile
from concourse import mybir

F32 = mybir.dt.float32
F32R = mybir.dt.float32r
BF16 = mybir.dt.bfloat16
FP8 = mybir.dt.float8e4
AF = mybir.ActivationFunctionType
OP = mybir.AluOpType

# ---------------------------------------------------------------------------
# walrus in this image rejects >1 sync-wait on a Drain (CTRL) instruction;
# split the TileContext epilogue drain into a chain of single-wait drains.
_orig_drain_and_barrier = tile.TileContext._drain_and_barrier


def _split_drain_and_barrier(self, tick_clock, wait_clock):
    from bass_rust import ScopedClock

    drain_inst = self.nc.sync.drain()
    wait_clock.add_sem_waits(drain_inst.ins, ScopedClock({None: tick_clock.global_clock}))
    waits = list(drain_inst.ins.sync_info.on_wait)
    if len(waits) > 1:
        ups = list(drain_inst.ins.sync_info.on_update)
        drain_inst.ins.sync_info = bass_rust.SyncInfo(on_wait=waits[:1], on_update=[])
        rest = waits[1:]
        while rest:
            chunk, rest = rest[:1], rest[1:]
            d2 = self.nc.sync.drain()
            d2.ins.sync_info = bass_rust.SyncInfo(
                on_wait=chunk, on_update=[] if rest else ups
            )
    self.nc.all_engine_barrier()
    assert self.sems is not None
    popped = self.nc._tile_sem_poison_stack.pop()
    assert popped is self._sem_poison
    self.nc.clear_and_free_semaphores(list(self.sems.allocated().values()))
    self.nc.all_engine_barrier()


tile.TileContext._drain_and_barrier = _split_drain_and_barrier

_WAIT_CAP = 1


def split_excess_waits(nc):
    """walrus in this image caps sync-waits per instruction very low. Move
    excess waits onto same-engine NOPs inserted immediately before the
    instruction (engine queues are in-order, so the gating is preserved)."""
    nid = [0]

    def mk_nop(engine, waits):
        nid[0] += 1
        n = bass_rust.InstNoOp(name=f"WSPL-{nid[0]}", engine=engine, ins=[], outs=[])
        n.sync_info = bass_rust.SyncInfo(on_wait=waits, on_update=[])
        return n

    for f in nc.m.functions:
        for bb in f.blocks:
            out = []
            for inst in bb.instructions:
                si = inst.sync_info
                waits = list(si.on_wait) if si is not None else []
                if len(waits) > _WAIT_CAP:
                    keep = waits[: _WAIT_CAP]
                    rest = waits[_WAIT_CAP:]
                    while rest:
                        chunk, rest = rest[:_WAIT_CAP], rest[_WAIT_CAP:]
                        out.append(mk_nop(inst.engine, chunk))
                    inst.sync_info = bass_rust.SyncInfo(
                        on_wait=keep, on_update=list(si.on_update))
                out.append(inst)
            bb.instructions = out


# ---------------------------------------------------------------------------
class Cfg:
    def __init__(self, DIM=1024, NB=2, NPB=2048, DH=64, H=2, IC=512, eps=1e-5):
        self.DIM, self.NB, self.NPB, self.DH, self.H = DIM, NB, NPB, DH, H
        self.TOK = NB * NPB
        self.KC = DIM // 128          # k-chunks of the QKV contraction
        self.QC = H * DH              # q/k/v columns per core (128)
        self.FC = 512                 # free chunk for matmuls
        self.QT = 1024                # tokens per quarter
        self.NQ = self.TOK // self.QT
        self.JT = NPB // 128          # key tiles per batch
        self.IC = IC                  # query chunk
        self.ICN = NPB // IC
        self.DO = DIM                 # out-proj output dim
        self.DOT = DIM // 128
        self.eps = eps
        assert self.QC == 128 and DIM % 128 == 0 and NPB % 128 == 0
        assert self.QT % self.FC == 0 and NPB % IC == 0 and NPB % self.QT == 0


def build_nc(c: Cfg, split_waits: bool = True):
    nc = bass.Bass("TRN2", target_bir_lowering=False)

    xt_d = nc.dram_tensor("xt", [128, c.KC, c.TOK], BF16, kind="ExternalInput")
    xsq_d = nc.dram_tensor("xsq", [128, c.KC, c.TOK], FP8, kind="ExternalInput")
    wq_d = nc.dram_tensor("wq", [128, c.KC, 3 * c.QC], BF16, kind="ExternalInput")
    fixc_d = nc.dram_tensor("fixc", [128, 6], F32, kind="ExternalInput")
    cosk_d = nc.dram_tensor("cosk", [128, c.NPB], BF16, kind="ExternalInput")
    sink_d = nc.dram_tensor("sink", [128, c.NPB], BF16, kind="ExternalInput")
    wout_d = nc.dram_tensor("wout", [128, c.DO], F32R, kind="ExternalInput")
    idblk_d = nc.dram_tensor("idblk", [128, c.DH], BF16, kind="ExternalInput")
    out_d = nc.dram_tensor("out", [128, c.DOT, c.TOK], F32, kind="ExternalOutput")

    FC = c.FC
    pairswap = [i ^ 1 for i in range(32)]

    with ExitStack() as ctx:
        tc = ctx.enter_context(tile.TileContext(nc))
        wp = ctx.enter_context(tc.tile_pool(name="wp", bufs=1))
        r1 = ctx.enter_context(tc.tile_pool(name="r1", bufs=1))
        rb = ctx.enter_context(tc.tile_pool(name="rb", bufs=2))
        r3 = ctx.enter_context(tc.tile_pool(name="r3", bufs=3))
        pp = ctx.enter_context(tc.tile_pool(name="pp", bufs=2, space="PSUM"))

        xts, xqs = {}, {}

        def dma_quarter(q):
            t = rb.tile([128, c.KC, c.QT], BF16, tag="xt", name=f"xtq{q}")
            s = rb.tile([128, c.KC, c.QT], FP8, tag="xq", name=f"xqq{q}")
            for i in range(4):
                kcs = slice(i * (c.KC // 4), (i + 1) * (c.KC // 4))
                tsl = slice(q * c.QT, (q + 1) * c.QT)
                nc.sync.dma_start(t[:, kcs, :], xt_d[:, kcs, tsl])
                nc.sync.dma_start(s[:, kcs, :], xsq_d[:, kcs, tsl])
            xts[q], xqs[q] = t, s

        dma_quarter(0)
        dma_quarter(1)
        wq = wp.tile([128, c.KC, 3 * c.QC], BF16)
        nc.sync.dma_start(wq[:], wq_d[:])
        ones_c = wp.tile([128, 1], BF16)
        nc.vector.memset(ones_c[:], 1.0)
        ones_8 = wp.tile([128, 1], FP8)
        nc.vector.memset(ones_8[:], 1.0)
        onesf = wp.tile([1, 128], F32)
        nc.vector.memset(onesf[:], 1.0)
        ones_r = wp.tile([1, 128], F32R)
        nc.vector.tensor_copy(ones_r[:], onesf[:])
        sel2f = wp.tile([1, 2, 128], F32)
        nc.vector.memset(sel2f[:], 0.0)
        nc.vector.memset(sel2f[:, 0, 0:64], 1.0)
        nc.vector.memset(sel2f[:, 1, 64:128], 1.0)
        sel2 = wp.tile([1, 2, 128], F32R)
        nc.vector.tensor_copy(sel2[:], sel2f[:])
        fixc = wp.tile([128, 6], F32)
        nc.sync.dma_start(fixc[:], fixc_d[:])
        cosk = wp.tile([128, c.NPB], BF16)
        nc.sync.dma_start(cosk[:], cosk_d[:])
        sink = wp.tile([128, c.NPB], BF16)
        nc.sync.dma_start(sink[:], sink_d[:])
        wout = wp.tile([128, c.DO], F32R)
        nc.sync.dma_start(wout[:], wout_d[:])
        idblk = wp.tile([128, c.DH], BF16)
        nc.sync.dma_start(idblk[:], idblk_d[:])

        bstate = {}
        pend = [None]   # deferred per-chunk epilogue state

        from collections import deque
        fillers = deque()

        def unit(fn):
            fillers.append(fn)

        def pump(n=2):
            for _ in range(min(n, len(fillers))):
                fillers.popleft()()

        def drain():
            while fillers:
                fillers.popleft()()

        def queue_stats(q):
            b, qt = divmod(q, 2)

            def alloc_batch():
                st = {
                    "qfix": r1.tile([128, c.NPB], F32, tag="qfix", name=f"qfix{b}"),
                    "kfix": r1.tile([128, c.NPB], F32, tag="kfix", name=f"kfix{b}"),
                    "vfix": r1.tile([128, c.NPB], BF16, tag="vfix", name=f"vfix{b}"),
                    "qbf": rb.tile([128, c.NPB], BF16, tag="qbf", name=f"qbf{b}"),
                    "kbf": rb.tile([128, c.NPB], BF16, tag="kbf", name=f"kbf{b}"),
                    "sb": [None, None], "mb": [None, None],
                }
                bstate[b] = st

            def alloc_stat():
                st = bstate[b]
                mun_q = rb.tile([1, c.QT], F32R, tag="mun", name=f"mun{q}")
                srow_q = rb.tile([1, c.QT], F32R, tag="srow", name=f"srow{q}")
                st[f"stat{qt}"] = (srow_q, mun_q)

            if qt == 0:
                unit(alloc_batch)
            unit(alloc_stat)

            def stat_mm(fc, which):
                def go():
                    xt_t, xq_t = xts[q], xqs[q]
                    fsl = slice(fc * FC, (fc + 1) * FC)
                    if which == 0:
                        s1P = pp.tile([1, FC], F32, tag="mm", name="s1P")
                        for kc in range(c.KC):
                            nc.tensor.matmul(s1P[:], ones_c[:], xt_t[:, kc, fsl],
                                             start=(kc == 0), stop=(kc == c.KC - 1))
                        bstate[b][f"s1P{qt}{fc}"] = s1P
                    else:
                        s2P = pp.tile([1, FC], F32, tag="mm", name="s2P")
                        for kc in range(c.KC):
                            nc.tensor.matmul(s2P[:], ones_8[:], xq_t[:, kc, fsl],
                                             start=(kc == 0), stop=(kc == c.KC - 1))
                        bstate[b][f"s2P{qt}{fc}"] = s2P
                return go

            def stat_math(fc):
                def go():
                    st = bstate[b]
                    srow_q, mun_q = st[f"stat{qt}"]
                    s1P, s2P = st[f"s1P{qt}{fc}"], st[f"s2P{qt}{fc}"]
                    fsl = slice(fc * FC, (fc + 1) * FC)
                    mun = mun_q[:, fsl]
                    nc.vector.tensor_scalar_mul(mun, s1P[:], -1.0 / c.DIM)
                    ve = r1.tile([1, FC], F32, tag="ve", name="ve")
                    nc.vector.tensor_scalar(ve[:], s2P[:], 1.0 / c.DIM, c.eps,
                                            OP.mult, OP.add)
                    m2 = r1.tile([1, FC], F32, tag="m2", name="m2")
                    nc.vector.tensor_tensor(m2[:], mun.bitcast(F32),
                                            mun.bitcast(F32), OP.mult)
                    vef = r1.tile([1, FC], F32, tag="vef", name="vef")
                    nc.vector.scalar_tensor_tensor(vef[:], m2[:], -1.0, ve[:],
                                                   OP.mult, OP.add)
                    lnv = r1.tile([1, FC], F32, tag="lnv", name="lnv")
                    nc.scalar.activation(lnv[:], vef[:], AF.Ln)
                    nc.scalar.activation(srow_q[:, fsl], lnv[:], AF.Exp,
                                         scale=-0.5)
                return go

            for fc in range(2):
                unit(stat_mm(fc, 0))
                unit(stat_mm(fc, 1))
                unit(stat_math(fc))

        def queue_bcqkv(q):
            b, qt = divmod(q, 2)

            def bc_unit():
                st = bstate[b]
                srow_q, mun_q = st[f"stat{qt}"]
                s_b = rb.tile([128, c.QT], F32, tag="sb", name=f"sb{q}")
                m_b = rb.tile([128, c.QT], F32, tag="mb", name=f"mb{q}")
                st["sb"][qt], st["mb"][qt] = s_b, m_b
                for fc in range(2):
                    fsl = slice(fc * FC, (fc + 1) * FC)
                    bcp = pp.tile([128, FC], F32, tag="mm", name="bcs")
                    nc.tensor.matmul(bcp[:], ones_r[:], srow_q[:, fsl],
                                     start=True, stop=True)
                    nc.scalar.copy(s_b[:, fsl], bcp[:])
                    bcm = pp.tile([128, FC], F32, tag="mm", name="bcm")
                    nc.tensor.matmul(bcm[:], ones_r[:], mun_q[:, fsl],
                                     start=True, stop=True)
                    nc.scalar.copy(m_b[:, fsl], bcm[:])

            unit(bc_unit)

            def qkv_unit(fc, m):
                def go():
                    st = bstate[b]
                    xt_t = xts[q]
                    m_b = st["mb"][qt]
                    fsl = slice(fc * FC, (fc + 1) * FC)
                    gsl = slice(qt * c.QT + fc * FC, qt * c.QT + (fc + 1) * FC)
                    dst = (st["qfix"], st["kfix"], st["vfix"])[m]
                    qp = pp.tile([128, FC], F32, tag="mm", name="qp")
                    for kc in range(c.KC):
                        nc.tensor.matmul(qp[:], wq[:, kc, m * 128:(m + 1) * 128],
                                         xt_t[:, kc, fsl],
                                         start=(kc == 0), stop=(kc == c.KC - 1))
                    nc.vector.scalar_tensor_tensor(dst[:, gsl], m_b[:, fsl],
                                                   fixc[:, m:m + 1], qp[:],
                                                   OP.mult, OP.add)
                return go

            for fc in range(2):
                for m in range(3):
                    unit(qkv_unit(fc, m))

            def vscale_unit():
                st = bstate[b]
                qsl = slice(qt * c.QT, (qt + 1) * c.QT)
                nc.vector.tensor_tensor(st["vfix"][:, qsl], st["vfix"][:, qsl],
                                        st["sb"][qt][:], OP.mult)

            unit(vscale_unit)

        def queue_batch_prep(b, half, what="all"):
            def alloc_unit():
                st = bstate[b]
                vaug = rb.tile([128, c.H, c.JT, c.DH + 1], BF16, tag="vaug",
                               name=f"vaug{b}")
                st["vaug"] = vaug
                nc.vector.memset(vaug[:, :, :, c.DH], 1.0)
                st["ctx"] = rb.tile([128, c.NPB], F32R, tag="ctx", name=f"ctx{b}")

            if half == 0 and what in ("vt", "all"):
                unit(alloc_unit)

            # V transpose into per-head [keys, dh+ones] tiles (ones row last
            # gives the softmax denominator as row DH of the attnV PSUM tile)
            def vt_unit(hh, half):
                def go():
                    st = bstate[b]
                    vfix, vaug = st["vfix"], st["vaug"]
                    dsl = slice(hh * c.DH, (hh + 1) * c.DH)
                    for jt in range(half * 8, half * 8 + 8):
                        tp = pp.tile([128, c.DH], BF16, tag="mm", name="vt")
                        nc.tensor.transpose(tp[:],
                                            vfix[dsl, jt * 128:(jt + 1) * 128],
                                            idblk[dsl, :])
                        if jt % 2 == 0:
                            nc.scalar.copy(vaug[:, hh, jt, 0:c.DH], tp[:])
                        else:
                            nc.vector.tensor_copy(vaug[:, hh, jt, 0:c.DH], tp[:])
                return go



            # RoPE in f32; the per-token LN scale and the bf16 rounding ride
            # the last op. (rotation commutes with the per-token scale)
            def rope_unit(which, hq):
                def go():
                    st = bstate[b]
                    src = st["qfix"] if which == 0 else st["kfix"]
                    dst = st["qbf"] if which == 0 else st["kbf"]
                    hsl = slice(hq * c.QT, (hq + 1) * c.QT)
                    sh = r1.tile([128, c.QT], F32, tag="sh", name="sh")
                    nc.vector.stream_shuffle(sh[:], src[:, hsl], pairswap)
                    nc.vector.tensor_tensor(src[:, hsl], src[:, hsl],
                                            cosk[:, hsl], OP.mult)
                    nc.vector.tensor_tensor(sh[:], sh[:], sink[:, hsl], OP.mult)
                    nc.vector.tensor_tensor(src[:, hsl], src[:, hsl], sh[:],
                                            OP.add)
                    nc.vector.tensor_tensor(dst[:, hsl], src[:, hsl],
                                            st["sb"][hq][:], OP.mult)
                return go

            if what in ("vt", "all"):
                unit(vt_unit(0, half))
                unit(vt_unit(1, half))
            if what in ("rope", "all"):
                unit(rope_unit(0, half))
                unit(rope_unit(1, half))

        def flush_pend_stage(stage):
            """Deferred epilogue of the previous query chunk, staged into the
            current chunk's jt loop so the PE never waits on the Act chain."""
            if pend[0] is None:
                return
            p = pend[0]
            if stage == "dlr":        # 1/denominator on Act (ln then exp)
                dl = r1.tile([1, 2 * c.IC], F32, tag="dl", name="dl")
                nc.scalar.activation(dl[:], p["ds"][:], AF.Ln)
                dr = r1.tile([1, 2 * c.IC], F32R, tag="dr", name="dr")
                nc.scalar.activation(dr[:], dl[:], AF.Exp, scale=-1.0)
                p["dr"] = dr
            elif stage == "rp":       # broadcast 1/denom over inner rows
                rp = pp.tile([128, c.IC], F32, tag="mm", name="rp")
                for hh in range(2):
                    nc.tensor.matmul(rp[:], sel2[:, hh, :],
                                     p["dr"][:, hh * c.IC:(hh + 1) * c.IC],
                                     start=(hh == 0), stop=(hh == 1))
                rps = rb.tile([128, c.IC], F32, tag="rps", bufs=1, name="rps")
                nc.vector.tensor_copy(rps[:], rp[:])
                # normalize the raw-evicted context in place
                ctx, isl = p["ctx"], p["isl"]
                nc.vector.tensor_tensor(ctx[0:64, isl],
                                        ctx[0:64, isl].bitcast(F32),
                                        rps[0:64, :], OP.mult)
                nc.vector.tensor_tensor(ctx[64:128, isl],
                                        ctx[64:128, isl].bitcast(F32),
                                        rps[64:128, :], OP.mult)
            elif isinstance(stage, int):   # out-proj, 2 dim-blocks per call
                ctx, isl = p["ctx"], p["isl"]
                hoff = p["b"] * c.NPB + p["ic"] * c.IC
                for mt in range(stage * 2, stage * 2 + 2):
                    yp = pp.tile([128, c.IC], F32, tag="mm", name="yp")
                    nc.tensor.matmul(yp[:], wout[:, mt * 128:(mt + 1) * 128],
                                     ctx[:, isl], start=True, stop=True)
                    yt = r3.tile([128, c.IC], F32, tag="yt", bufs=2, name="yt")
                    nc.vector.tensor_copy(yt[:], yp[:])
                    nc.sync.dma_start(out_d[:, mt, hoff:hoff + c.IC], yt[:])
                if stage == 3:
                    pend[0] = None

        def emit_attention(b, ic):
            st = bstate[b]
            qbf, kbf, vaug, ctx = st["qbf"], st["kbf"], st["vaug"], st["ctx"]
            isl = slice(ic * c.IC, (ic + 1) * c.IC)
            # free the previous chunk's attnV accumulators: raw-evict context
            # (normalization happens later, off the critical path)
            if pend[0] is not None:
                p = pend[0]
                nc.vector.tensor_copy(p["ctx"][0:64, p["isl"]], p["cps0"][0:64, :])
                nc.vector.tensor_copy(p["ctx"][64:128, p["isl"]], p["cps1"][0:64, :])
            cps0 = pp.tile([128, c.IC], F32, tag="c0", bufs=1, name="cps0")
            cps1 = pp.tile([128, c.IC], F32, tag="c1", bufs=1, name="cps1")

            def emit_scores(jt):
                jsl = slice(jt * 128, (jt + 1) * 128)
                sp = pp.tile([128, 2 * c.IC], F32, tag="sp", name="sp")
                nc.tensor.matmul(sp[:, 0:c.IC], kbf[0:64, jsl], qbf[0:64, isl],
                                 start=True, stop=True, tile_position=(0, 0))
                nc.tensor.matmul(sp[:, c.IC:], kbf[64:128, jsl],
                                 qbf[64:128, isl],
                                 start=True, stop=True, tile_position=(64, 0))
                return sp

            sps = {0: emit_scores(0), 1: emit_scores(1)}
            for jt in range(c.JT):
                sp = sps.pop(jt)
                e = r3.tile([128, 2 * c.IC], BF16, tag="e", bufs=2, name="e")
                nc.scalar.activation(e[:], sp[:], AF.Exp)
                if jt == 1:
                    flush_pend_stage("dlr")
                if jt + 2 < c.JT:
                    sps[jt + 2] = emit_scores(jt + 2)
                if jt == 4:
                    flush_pend_stage("rp")
                elif 5 <= jt <= 8:
                    flush_pend_stage(jt - 5)
                nc.tensor.matmul(cps0[0:c.DH + 1, :], vaug[:, 0, jt, :],
                                 e[:, 0:c.IC], start=(jt == 0), stop=(jt == c.JT - 1))
                nc.tensor.matmul(cps1[0:c.DH + 1, :], vaug[:, 1, jt, :],
                                 e[:, c.IC:], start=(jt == 0), stop=(jt == c.JT - 1))
                if 7 <= jt <= 13:
                    pump(3)
                elif jt < 7 and pend[0] is None:
                    pump(2)
            # softmax denominators out of the accumulators (row DH)
            ds = r1.tile([1, 2 * c.IC], F32, tag="ds", name="ds")
            nc.vector.tensor_copy(ds[:, 0:c.IC], cps0[c.DH:c.DH + 1, :])
            nc.vector.tensor_copy(ds[:, c.IC:], cps1[c.DH:c.DH + 1, :])
            pend[0] = {"b": b, "ic": ic, "isl": isl, "ctx": ctx,
                       "cps0": cps0, "cps1": cps1, "ds": ds}

        def flush_tail():
            p = pend[0]
            nc.vector.tensor_copy(p["ctx"][0:64, p["isl"]], p["cps0"][0:64, :])
            nc.vector.tensor_copy(p["ctx"][64:128, p["isl"]], p["cps1"][0:64, :])
            flush_pend_stage("dlr")
            flush_pend_stage("rp")
            for s in range(4):
                flush_pend_stage(s)

        # ---- global schedule ----
        # Lead-in: quarter-0 chain, with quarter-1 stat matmuls filling the
        # PE while quarter-0's stat math runs on DVE/Act; batch-0 half-0
        # RoPE/V-prep lands before attention starts. Everything for the
        # second half / second batch is pumped as filler units inside the
        # attention jt-loops so the PE never drains.
        queue_stats(0)
        queue_stats(1)
        drain()
        queue_bcqkv(0)
        drain()
        queue_batch_prep(0, 0, "vt")
        queue_bcqkv(1)
        queue_batch_prep(0, 0, "rope")
        queue_batch_prep(0, 1, "vt")
        queue_batch_prep(0, 1, "rope")
        drain()
        unit(lambda: dma_quarter(2))
        queue_stats(2)
        queue_bcqkv(2)
        queue_batch_prep(1, 0, "vt")
        queue_batch_prep(1, 0, "rope")
        unit(lambda: dma_quarter(3))
        queue_stats(3)
        queue_bcqkv(3)
        queue_batch_prep(1, 1, "vt")
        queue_batch_prep(1, 1, "rope")
        for ic in range(c.ICN):
            emit_attention(0, ic)
        drain()
        for ic in range(c.ICN):
            emit_attention(1, ic)
        flush_tail()

    if split_waits:
        split_excess_waits(nc)
    nc.finalize()
    return nc


# ---------------------------------------------------------------------------
# host side
def host_inputs(c: Cfg, core: int, x, ln_w, ln_b, w_qkv, w_out, b_out):
    """Build the per-core input dict (all numpy, layouts described in build_nc)."""
    DIM, DH, H = c.DIM, c.DH, c.H
    INNER = w_qkv.shape[1] // 3
    TOK = c.TOK
    bf = ml_dtypes.bfloat16
    f32 = np.float32
    xf = x.reshape(TOK, DIM)
    xt = np.ascontiguousarray(xf.T.reshape(c.KC, 128, TOK).transpose(1, 0, 2))
    xtb = xt.astype(bf)
    xsq = (xtb.astype(np.float32) ** 2).astype(ml_dtypes.float8_e4m3)

    cs = core * c.QC
    sc = DH ** -0.5
    wsl = np.concatenate([w_qkv[:, cs:cs + c.QC] * sc,
                          w_qkv[:, INNER + cs:INNER + cs + c.QC],
                          w_qkv[:, 2 * INNER + cs:2 * INNER + cs + c.QC]], axis=1)
    wq = (ln_w[:, None] * wsl).reshape(c.KC, 128, 3 * c.QC).transpose(1, 0, 2)
    u = ln_w @ wsl   # [384]
    v = ln_b @ wsl
    fixc = np.concatenate([u.reshape(3, c.QC).T, v.reshape(3, c.QC).T], axis=1)

    inv = np.exp(np.arange(0, DH, 2, dtype=np.float64) * (-np.log(10000.0) / DH))
    ang = np.arange(c.NPB, dtype=np.float64)[:, None] * inv[None, :]
    cosR = np.repeat(np.cos(ang), 2, axis=1)
    sinR = np.repeat(np.sin(ang), 2, axis=1)
    sign = np.tile([-1.0, 1.0], DH // 2)
    sinS = sinR * sign[None, :]
    cosk = np.tile(cosR.T, (H, 1))
    sink = np.tile(sinS.T, (H, 1))

    wout = w_out[core * 128:(core + 1) * 128, :]
    idblk = np.zeros((128, DH), np.float32)
    for i in range(DH):
        idblk[i, i] = 1.0
        idblk[DH + i, i] = 1.0

    return {
        "xt": xtb, "xsq": xsq, "wq": wq.astype(bf), "fixc": fixc.astype(f32),
        "cosk": cosk.astype(bf), "sink": sink.astype(bf),
        "wout": wout.astype(f32), "idblk": idblk.astype(bf),
    }


def assemble_output(c: Cfg, outs, B, N, b_out=None):
    yT = np.zeros((c.DO, c.TOK), np.float64)
    for o in outs:
        yT += o.transpose(1, 0, 2).reshape(c.DO, c.TOK).astype(np.float64)
    y = np.ascontiguousarray(yT.T.reshape(B, N, c.DIM))
    if b_out is not None:
        y += b_out.astype(np.float64)
    return y.astype(np.float32)


_NC_CACHE = {}


def kernel(x, ln_w, ln_b, w_qkv, w_out, b_out):
    from concourse.bass_utils import run_bass_kernel_spmd

    x = np.asarray(x, np.float32)
    ln_w = np.asarray(ln_w, np.float32)
    ln_b = np.asarray(ln_b, np.float32)
    w_qkv = np.asarray(w_qkv, np.float32)
    w_out = np.asarray(w_out, np.float32)
    b_out = np.asarray(b_out, np.float32)
    assert np.allclose(ln_b, 0.0), "device fast-path assumes ln_b == 0"

    B, N, DIM = x.shape
    c = Cfg(DIM=DIM, NB=B, NPB=N)
    key = (DIM, B, N)
    if key not in _NC_CACHE:
        _NC_CACHE[key] = build_nc(c)
    nc = _NC_CACHE[key]
    in_maps = [host_inputs(c, core, x, ln_w, ln_b, w_qkv, w_out, b_out)
               for core in range(8)]
    import time as _time
    last = None
    for attempt in range(3):
        try:
            res = run_bass_kernel_spmd(nc, in_maps, core_ids=list(range(8)))
            break
        except Exception as e:  # transient device-unrecoverable wedges recover on retry
            last = e
            _time.sleep(15)
    else:
        raise last
    return assemble_output(c, [res.results[cc]["out"] for cc in range(8)], B, N, b_out)
